# revision 3
# baseline (speedup 1.0000x reference)
# Trainium2 Bass kernel for nn_CrossAttention (B=2, Nq=4096, Nk=2048, D=128,
# Dv=768, H=4, hd=32).
#
# Sharding: data-parallel over (B x Nq-blocks): core c handles batch c//4,
# query rows (c%4)*1024 .. +1024. K/V/weights replicated per core.
#
# Math (host-folded):
#   qn = (q * rstd_q) @ WqT_eff + bq/sqrt(hd)   with WqT_eff = diag(rms_q_w) Wq^T / sqrt(hd)
#   kn = (k * rstd_k) @ WkT_eff + bk            with WkT_eff = diag(rms_k_w) Wk^T
#   S_h = qn_h kn_h^T  (scale already folded into q side)
#   A   = sum_h exp(S_h) / rowsum_h(exp S_h)    (no max subtraction: |S| < 8)
#   out = A @ (0.25 * V)
import numpy as np

B, NQ, NK, D, DV = 2, 4096, 2048, 128, 768
H, HD = 4, 32
N_CORES = 8
NQC = NQ * B // N_CORES  # 1024 queries per core
NQT = NQC // 128  # 8 query tiles per core
NKT = NK // 128  # 16 key tiles
RMS_EPS = 1.1920929e-07

_CACHE = {}


def _build_nc():
    import concourse.bacc as bacc
    import concourse.mybir as mybir
    import concourse.tile as tile

    fp32 = mybir.dt.float32
    f16 = mybir.dt.float16

    nc = bacc.Bacc("TRN2", target_bir_lowering=False, debug=False)

    q_d = nc.dram_tensor("q", [NQC, D], fp32, kind="ExternalInput").ap()
    k_d = nc.dram_tensor("k", [NK, D], fp32, kind="ExternalInput").ap()
    v_d = nc.dram_tensor("v", [NK, DV], f16, kind="ExternalInput").ap()
    wq_d = nc.dram_tensor("wqt", [D, D], f16, kind="ExternalInput").ap()
    wk_d = nc.dram_tensor("wkt", [D, D], f16, kind="ExternalInput").ap()
    bq_d = nc.dram_tensor("bqe", [D], fp32, kind="ExternalInput").ap()
    bk_d = nc.dram_tensor("bke", [D], fp32, kind="ExternalInput").ap()
    o_d = nc.dram_tensor("o", [NQC, DV], fp32, kind="ExternalOutput").ap()

    with tile.TileContext(nc) as tc:
        _tile_kernel(tc, o_d, q_d, k_d, v_d, wq_d, wk_d, bq_d, bk_d)
    nc.compile()
    return nc


def _tile_kernel(tc, o_d, q_d, k_d, v_d, wq_d, wk_d, bq_d, bk_d):
    from contextlib import ExitStack

    import concourse.mybir as mybir

    nc = tc.nc
    fp32 = mybir.dt.float32
    f16 = mybir.dt.float16
    AF = mybir.ActivationFunctionType
    OP = mybir.AluOpType

    ctx = ExitStack()
    with ctx:
        singles = ctx.enter_context(tc.tile_pool(name="singles", bufs=1))

        # resident tensors
        v_sb = singles.tile([128, NKT, DV], f16)
        nc.sync.dma_start(out=v_sb, in_=v_d.rearrange("(c p) d -> p c d", p=128))
        wq_sb = singles.tile([128, D], f16)
        nc.sync.dma_start(out=wq_sb, in_=wq_d)
        wk_sb = singles.tile([128, D], f16)
        nc.sync.dma_start(out=wk_sb, in_=wk_d)
        bq_sb = singles.tile([128, 1], fp32)
        nc.sync.dma_start(out=bq_sb, in_=bq_d[:, None])
        bk_sb = singles.tile([128, 1], fp32)
        nc.sync.dma_start(out=bk_sb, in_=bk_d[:, None])

        qx_sb = singles.tile([128, NQT, D], fp32)
        nc.sync.dma_start(out=qx_sb, in_=q_d.rearrange("(c p) d -> p c d", p=128))
        kx_sb = singles.tile([128, NKT, D], fp32)
        nc.sync.dma_start(out=kx_sb, in_=k_d.rearrange("(c p) d -> p c d", p=128))

        eps_sb = singles.tile([128, 1], fp32)
        nc.vector.memset(eps_sb, RMS_EPS)

        qxT = singles.tile([128, NQC], f16)  # normalized, transposed [d, tok]
        kxT = singles.tile([128, NK], f16)
        qT = singles.tile([128, NQC], f16)  # projected (per-head rows 32h..32h+31)
        kT = singles.tile([128, NK], f16)

        # ---- preamble: RMSNorm + transpose + projections ----
        with (
            tc.tile_pool(name="pre", bufs=3) as pre,
            tc.tile_pool(name="prepsum", bufs=2, space="PSUM") as prepsum,
        ):
            for x_sb, nt, xT in ((qx_sb, NQT, qxT), (kx_sb, NKT, kxT)):
                for t in range(nt):
                    xt = x_sb[:, t, :]
                    sq = pre.tile([128, D], fp32, tag="sq")
                    ssq = pre.tile([128, 1], fp32, tag="ssq")
                    nc.scalar.activation(sq, xt, AF.Square, accum_out=ssq)
                    sd = pre.tile([128, 1], fp32, tag="sd")
                    # sd = sqrt(mean(x^2) + eps)
                    nc.scalar.activation(
                        sd, ssq, AF.Sqrt, bias=eps_sb, scale=1.0 / D
                    )
                    rstd = pre.tile([128, 1], fp32, tag="rstd")
                    nc.vector.reciprocal(rstd, sd)
                    xn = pre.tile([128, D], f16, tag="xn")
                    nc.vector.tensor_scalar_mul(xn, xt, rstd)
                    nc.sync.dma_start_transpose(
                        out=xT[:, t * 128 : (t + 1) * 128], in_=xn
                    )
            for xT, n512, w_sb, b_sb, dst in (
                (qxT, NQC // 512, wq_sb, bq_sb, qT),
                (kxT, NK // 512, wk_sb, bk_sb, kT),
            ):
                for j in range(n512):
                    pp = prepsum.tile([128, 512], fp32, tag="proj")
                    nc.tensor.matmul(
                        pp,
                        lhsT=w_sb,
                        rhs=xT[:, j * 512 : (j + 1) * 512],
                        start=True,
                        stop=True,
                    )
                    nc.scalar.activation(
                        dst[:, j * 512 : (j + 1) * 512], pp, AF.Identity, bias=b_sb
                    )

        # ---- main loop over query tiles ----
        with (
            tc.tile_pool(name="spsum", bufs=1, space="PSUM") as spool,
            tc.tile_pool(name="opsum", bufs=2, space="PSUM") as opool,
            tc.tile_pool(name="pwork", bufs=2) as pwork,
            tc.tile_pool(name="awork", bufs=2) as awork,
            tc.tile_pool(name="owork", bufs=2) as owork,
            tc.tile_pool(name="small", bufs=2) as small,
        ):
            for qc in range(NQT):
                qsl = slice(qc * 128, (qc + 1) * 128)
                P = pwork.tile([128, H, NK], f16, tag="P")
                racc = small.tile([128, 2, H], fp32, tag="racc")
                for pair in range(2):
                    for khalf in range(2):
                        stiles = []
                        for hh in range(2):
                            h = pair * 2 + hh
                            S = spool.tile([128, 1024], fp32, tag=f"S{hh}")
                            for kc in range(2):
                                koff = khalf * 1024 + kc * 512
                                nc.tensor.matmul(
                                    S[:, kc * 512 : (kc + 1) * 512],
                                    lhsT=qT[32 * h : 32 * (h + 1), qsl],
                                    rhs=kT[32 * h : 32 * (h + 1), koff : koff + 512],
                                    start=True,
                                    stop=True,
                                    tile_position=(32 * h, 0),
                                )
                            stiles.append((h, S))
                        for h, S in stiles:
                            nc.scalar.activation(
                                P[:, h, khalf * 1024 : (khalf + 1) * 1024],
                                S,
                                AF.Exp,
                                accum_out=racc[:, khalf, h : h + 1],
                            )
                # rowsums -> 1/R
                rsum = small.tile([128, H], fp32, tag="rsum")
                nc.vector.tensor_tensor(rsum, racc[:, 0, :], racc[:, 1, :], OP.add)
                crec = small.tile([128, H], fp32, tag="crec")
                nc.vector.reciprocal(crec, rsum)
                # A = sum_h P_h / R_h
                A = awork.tile([128, NK], f16, tag="A")
                nc.vector.tensor_scalar_mul(A, P[:, 0, :], crec[:, 0:1])
                for h in range(1, H):
                    nc.vector.scalar_tensor_tensor(
                        out=A,
                        in0=P[:, h, :],
                        scalar=crec[:, h : h + 1],
                        in1=A,
                        op0=OP.mult,
                        op1=OP.add,
                    )
                # A^T via DMA xbar transpose, 128x128 blocks
                AT = awork.tile([128, NK], f16, tag="AT")
                for kc in range(NKT):
                    nc.sync.dma_start_transpose(
                        out=AT[:, kc * 128 : (kc + 1) * 128],
                        in_=A[:, kc * 128 : (kc + 1) * 128],
                    )
                # O = A @ V  (V pre-scaled by 0.25)
                O = opool.tile([128, DV], fp32, tag="O")
                for kc in range(NKT):
                    st, sp = kc == 0, kc == NKT - 1
                    lhsT = AT[:, kc * 128 : (kc + 1) * 128]
                    nc.tensor.matmul(
                        O[:, 0:512], lhsT=lhsT, rhs=v_sb[:, kc, 0:512],
                        start=st, stop=sp,
                    )
                    nc.tensor.matmul(
                        O[:, 512:DV], lhsT=lhsT, rhs=v_sb[:, kc, 512:DV],
                        start=st, stop=sp,
                    )
                osb = owork.tile([128, DV], fp32, tag="osb")
                nc.vector.tensor_copy(osb, O)
                nc.sync.dma_start(out=o_d[qsl, :], in_=osb)


def _get_nc():
    if "nc" not in _CACHE:
        _CACHE["nc"] = _build_nc()
    return _CACHE["nc"]


def _host_prep(query, key, value, rms_q_w, rms_k_w, Wq, Wk, bq, bk):
    s = np.sqrt(float(HD))
    wqt = (rms_q_w[:, None] * Wq.T / s).astype(np.float16)
    wkt = (rms_k_w[:, None] * Wk.T).astype(np.float16)
    bqe = (bq / s).astype(np.float32)
    bke = bk.astype(np.float32)
    vq = (0.25 * value).astype(np.float16)  # [B, NK, DV]
    in_maps = []
    nq_blk = NQ // (N_CORES // B)  # 1024
    for c in range(N_CORES):
        b, qi = divmod(c, N_CORES // B)
        in_maps.append(
            {
                "q": np.ascontiguousarray(
                    query[b, qi * nq_blk : (qi + 1) * nq_blk]
                ).astype(np.float32),
                "k": np.ascontiguousarray(key[b]).astype(np.float32),
                "v": np.ascontiguousarray(vq[b]),
                "wqt": wqt,
                "wkt": wkt,
                "bqe": bqe,
                "bke": bke,
            }
        )
    return in_maps


def kernel(query, key, value, rms_q_w, rms_k_w, Wq, Wk, bq, bk, _trace=False):
    from concourse import bass_utils

    query = np.asarray(query)
    key = np.asarray(key)
    value = np.asarray(value)
    in_maps = _host_prep(
        np.asarray(query), np.asarray(key), np.asarray(value),
        np.asarray(rms_q_w), np.asarray(rms_k_w),
        np.asarray(Wq), np.asarray(Wk), np.asarray(bq), np.asarray(bk),
    )
    nc = _get_nc()
    res = bass_utils.run_bass_kernel_spmd(
        nc, in_maps, core_ids=list(range(N_CORES)), trace=_trace
    )
    _CACHE["last_results"] = res
    outs = [np.asarray(r["o"], dtype=np.float32) for r in res.results]
    nq_blk = NQ // (N_CORES // B)
    out = np.empty((B, NQ, DV), dtype=np.float32)
    for c in range(N_CORES):
        b, qi = divmod(c, N_CORES // B)
        out[b, qi * nq_blk : (qi + 1) * nq_blk] = outs[c]
    return out


# revision 4
# speedup vs baseline: 1.2908x; 1.2908x over previous
# Trainium2 Bass kernel for nn_CrossAttention (B=2, Nq=4096, Nk=2048, D=128,
# Dv=768, H=4, hd=32).
#
# Sharding: data-parallel over (B x Nq-blocks): core c handles batch c//4,
# query rows (c%4)*1024 .. +1024. K/V/weights replicated per core.
#
# Math (host-folded):
#   qn = (q * rstd_q) @ WqT_eff + bq/sqrt(hd)   with WqT_eff = diag(rms_q_w) Wq^T / sqrt(hd)
#   kn = (k * rstd_k) @ WkT_eff + bk            with WkT_eff = diag(rms_k_w) Wk^T
#   S_h = qn_h kn_h^T  (scale already folded into q side)
#   A   = sum_h exp(S_h) / rowsum_h(exp S_h)    (no max subtraction: |S| < 8)
#   out = A @ (0.25 * V)
import numpy as np

B, NQ, NK, D, DV = 2, 4096, 2048, 128, 768
H, HD = 4, 32
N_CORES = 8
NQC = NQ * B // N_CORES  # 1024 queries per core
NQT = NQC // 128  # 8 query tiles per core
NKT = NK // 128  # 16 key tiles
RMS_EPS = 1.1920929e-07

_CACHE = {}


def _build_nc():
    import concourse.bacc as bacc
    import concourse.mybir as mybir
    import concourse.tile as tile

    fp32 = mybir.dt.float32
    f16 = mybir.dt.float16

    nc = bacc.Bacc("TRN2", target_bir_lowering=False, debug=False)

    q_d = nc.dram_tensor("q", [NQC, D], fp32, kind="ExternalInput").ap()
    k_d = nc.dram_tensor("k", [NK, D], fp32, kind="ExternalInput").ap()
    v_d = nc.dram_tensor("v", [NK, DV], f16, kind="ExternalInput").ap()
    wq_d = nc.dram_tensor("wqt", [D, D], f16, kind="ExternalInput").ap()
    wk_d = nc.dram_tensor("wkt", [D, D], f16, kind="ExternalInput").ap()
    bq_d = nc.dram_tensor("bqe", [D], fp32, kind="ExternalInput").ap()
    bk_d = nc.dram_tensor("bke", [D], fp32, kind="ExternalInput").ap()
    o_d = nc.dram_tensor("o", [NQC, DV], fp32, kind="ExternalOutput").ap()

    with tile.TileContext(nc) as tc:
        _tile_kernel(tc, o_d, q_d, k_d, v_d, wq_d, wk_d, bq_d, bk_d)
    nc.compile()
    return nc


def _tile_kernel(tc, o_d, q_d, k_d, v_d, wq_d, wk_d, bq_d, bk_d):
    from contextlib import ExitStack

    import concourse.mybir as mybir

    nc = tc.nc
    fp32 = mybir.dt.float32
    f16 = mybir.dt.float16
    AF = mybir.ActivationFunctionType
    OP = mybir.AluOpType
    AX = mybir.AxisListType

    ctx = ExitStack()
    with ctx:
        singles = ctx.enter_context(tc.tile_pool(name="singles", bufs=1))

        # resident tensors
        v_sb = singles.tile([128, NKT, DV], f16)
        nc.sync.dma_start(out=v_sb, in_=v_d.rearrange("(c p) d -> p c d", p=128))
        wq_sb = singles.tile([128, D], f16)
        nc.sync.dma_start(out=wq_sb, in_=wq_d)
        wk_sb = singles.tile([128, D], f16)
        nc.sync.dma_start(out=wk_sb, in_=wk_d)
        bq_sb = singles.tile([128, 1], fp32)
        nc.sync.dma_start(out=bq_sb, in_=bq_d[:, None])
        bk_sb = singles.tile([128, 1], fp32)
        nc.sync.dma_start(out=bk_sb, in_=bk_d[:, None])

        qx_sb = singles.tile([128, NQT, D], fp32)
        nc.sync.dma_start(out=qx_sb, in_=q_d.rearrange("(c p) d -> p c d", p=128))
        kx_sb = singles.tile([128, NKT, D], fp32)
        nc.sync.dma_start(out=kx_sb, in_=k_d.rearrange("(c p) d -> p c d", p=128))

        eps_sb = singles.tile([128, 1], fp32)
        nc.vector.memset(eps_sb, RMS_EPS)

        qxT = singles.tile([128, NQC], f16)  # normalized, transposed [d, tok]
        kxT = singles.tile([128, NK], f16)
        qT = singles.tile([128, NQC], f16)  # projected (per-head rows 32h..32h+31)
        kT = singles.tile([128, NK], f16)

        # ---- preamble: RMSNorm + transpose + projections ----
        with (
            tc.tile_pool(name="pre", bufs=2) as pre,
            tc.tile_pool(name="prepsum", bufs=2, space="PSUM") as prepsum,
        ):
            for x_sb, nt, xT in ((qx_sb, NQT, qxT), (kx_sb, NKT, kxT)):
                n = nt * D
                sq = pre.tile([128, nt, D], fp32, tag="sq", bufs=1)
                nc.vector.tensor_mul(sq, x_sb, x_sb)
                ssq = pre.tile([128, nt], fp32, tag="ssq", bufs=1)
                nc.vector.tensor_reduce(ssq[:, :, None], sq, AX.X, OP.add)
                sd = pre.tile([128, nt], fp32, tag="sd", bufs=1)
                # sd = sqrt(mean(x^2) + eps)  (bias AP is per-partition)
                nc.scalar.activation(sd, ssq, AF.Sqrt, bias=eps_sb, scale=1.0 / D)
                rstd = pre.tile([128, nt], fp32, tag="rstd", bufs=1)
                nc.vector.reciprocal(rstd, sd)
                xn = pre.tile([128, nt, D], f16, tag="xn", bufs=1)
                for t in range(nt):
                    nc.vector.tensor_scalar_mul(
                        xn[:, t, :], x_sb[:, t, :], rstd[:, t : t + 1]
                    )
                # one xbar transpose: out[p, c, j] = xn_flat[j, c*128+p]
                # -> xT block c holds (xn tile c)^T, i.e. xT[d, tok] overall.
                nc.sync.dma_start_transpose(
                    out=xT.rearrange("p (c j) -> p c j", j=128),
                    in_=xn.rearrange("p c j -> p (c j)"),
                )
            for xT, n512, w_sb, b_sb, dst in (
                (qxT, NQC // 512, wq_sb, bq_sb, qT),
                (kxT, NK // 512, wk_sb, bk_sb, kT),
            ):
                for j in range(n512):
                    pp = prepsum.tile([128, 512], fp32, tag="proj")
                    nc.tensor.matmul(
                        pp,
                        lhsT=w_sb,
                        rhs=xT[:, j * 512 : (j + 1) * 512],
                        start=True,
                        stop=True,
                    )
                    nc.scalar.activation(
                        dst[:, j * 512 : (j + 1) * 512], pp, AF.Identity, bias=b_sb
                    )

        # ---- main loop over query tiles ----
        with (
            tc.tile_pool(name="spsum", bufs=1, space="PSUM") as spool,
            tc.tile_pool(name="opsum", bufs=2, space="PSUM") as opool,
            tc.tile_pool(name="pwork", bufs=2) as pwork,
            tc.tile_pool(name="awork", bufs=2) as awork,
            tc.tile_pool(name="owork", bufs=2) as owork,
            tc.tile_pool(name="small", bufs=2) as small,
        ):
            for qc in range(NQT):
                qsl = slice(qc * 128, (qc + 1) * 128)
                P = pwork.tile([128, H, NK], f16, tag="P")
                racc = small.tile([128, H], fp32, tag="racc")
                for h in range(H):
                    S = spool.tile([128, NK], fp32, tag="S")  # 4 banks
                    for kc in range(NK // 512):
                        nc.tensor.matmul(
                            S[:, kc * 512 : (kc + 1) * 512],
                            lhsT=qT[32 * h : 32 * (h + 1), qsl],
                            rhs=kT[32 * h : 32 * (h + 1), kc * 512 : (kc + 1) * 512],
                            start=True,
                            stop=True,
                            tile_position=(32 * h, 0),
                        )
                    nc.scalar.activation(
                        P[:, h, :], S, AF.Exp, accum_out=racc[:, h : h + 1]
                    )
                # 1/R per head
                crec = small.tile([128, H], fp32, tag="crec")
                nc.vector.reciprocal(crec, racc)
                # A = sum_h P_h / R_h   (TS at 4x + TT tree-adds at 2x)
                t1 = awork.tile([128, NK], f16, tag="t1")
                t2 = awork.tile([128, NK], f16, tag="t2")
                t3 = awork.tile([128, NK], f16, tag="t3")
                A = awork.tile([128, NK], f16, tag="A")
                nc.vector.tensor_scalar_mul(A, P[:, 0, :], crec[:, 0:1])
                nc.vector.tensor_scalar_mul(t1, P[:, 1, :], crec[:, 1:2])
                nc.vector.tensor_scalar_mul(t2, P[:, 2, :], crec[:, 2:3])
                nc.vector.tensor_scalar_mul(t3, P[:, 3, :], crec[:, 3:4])
                nc.vector.tensor_add(t2, t2, t3)
                nc.vector.tensor_add(A, A, t1)
                nc.vector.tensor_add(A, A, t2)
                # A^T in one xbar transpose: AT block kc = (A cols kc*128..)^T
                AT = awork.tile([128, NK], f16, tag="AT")
                nc.sync.dma_start_transpose(
                    out=AT.rearrange("p (c j) -> p c j", j=128), in_=A
                )
                # O = A @ V  (V pre-scaled by 0.25)
                O = opool.tile([128, DV], fp32, tag="O")
                for kc in range(NKT):
                    st, sp = kc == 0, kc == NKT - 1
                    lhsT = AT[:, kc * 128 : (kc + 1) * 128]
                    nc.tensor.matmul(
                        O[:, 0:512], lhsT=lhsT, rhs=v_sb[:, kc, 0:512],
                        start=st, stop=sp,
                    )
                    nc.tensor.matmul(
                        O[:, 512:DV], lhsT=lhsT, rhs=v_sb[:, kc, 512:DV],
                        start=st, stop=sp,
                    )
                osb = owork.tile([128, DV], fp32, tag="osb")
                nc.vector.tensor_copy(osb, O)
                nc.sync.dma_start(out=o_d[qsl, :], in_=osb)


def _get_nc():
    if "nc" not in _CACHE:
        _CACHE["nc"] = _build_nc()
    return _CACHE["nc"]


def _host_prep(query, key, value, rms_q_w, rms_k_w, Wq, Wk, bq, bk):
    s = np.sqrt(float(HD))
    wqt = (rms_q_w[:, None] * Wq.T / s).astype(np.float16)
    wkt = (rms_k_w[:, None] * Wk.T).astype(np.float16)
    bqe = (bq / s).astype(np.float32)
    bke = bk.astype(np.float32)
    vq = (0.25 * value).astype(np.float16)  # [B, NK, DV]
    in_maps = []
    nq_blk = NQ // (N_CORES // B)  # 1024
    for c in range(N_CORES):
        b, qi = divmod(c, N_CORES // B)
        in_maps.append(
            {
                "q": np.ascontiguousarray(
                    query[b, qi * nq_blk : (qi + 1) * nq_blk]
                ).astype(np.float32),
                "k": np.ascontiguousarray(key[b]).astype(np.float32),
                "v": np.ascontiguousarray(vq[b]),
                "wqt": wqt,
                "wkt": wkt,
                "bqe": bqe,
                "bke": bke,
            }
        )
    return in_maps


def kernel(query, key, value, rms_q_w, rms_k_w, Wq, Wk, bq, bk, _trace=False):
    from concourse import bass_utils

    in_maps = _host_prep(
        np.asarray(query), np.asarray(key), np.asarray(value),
        np.asarray(rms_q_w), np.asarray(rms_k_w),
        np.asarray(Wq), np.asarray(Wk), np.asarray(bq), np.asarray(bk),
    )
    nc = _get_nc()
    res = bass_utils.run_bass_kernel_spmd(
        nc, in_maps, core_ids=list(range(N_CORES)), trace=_trace
    )
    _CACHE["last_results"] = res
    outs = [np.asarray(r["o"], dtype=np.float32) for r in res.results]
    nq_blk = NQ // (N_CORES // B)
    out = np.empty((B, NQ, DV), dtype=np.float32)
    for c in range(N_CORES):
        b, qi = divmod(c, N_CORES // B)
        out[b, qi * nq_blk : (qi + 1) * nq_blk] = outs[c]
    return out


# revision 5
# speedup vs baseline: 1.6499x; 1.2782x over previous
# Trainium2 Bass kernel for nn_CrossAttention (B=2, Nq=4096, Nk=2048, D=128,
# Dv=768, H=4, hd=32).
#
# Sharding: data-parallel over (B x Nq-blocks): core c handles batch c//4,
# query rows (c%4)*1024 .. +1024. K/V/weights replicated per core.
#
# Math (host-folded):
#   qn = (q * rstd_q) @ WqT_eff + bq/sqrt(hd)   with WqT_eff = diag(rms_q_w) Wq^T / sqrt(hd)
#   kn = (k * rstd_k) @ WkT_eff + bk            with WkT_eff = diag(rms_k_w) Wk^T
#   S_h = qn_h kn_h^T  (scale already folded into q side)
#   A   = sum_h exp(S_h) / rowsum_h(exp S_h)    (no max subtraction: |S| < 8)
#   out = A @ (0.25 * V)
import numpy as np

B, NQ, NK, D, DV = 2, 4096, 2048, 128, 768
H, HD = 4, 32
N_CORES = 8
NQC = NQ * B // N_CORES  # 1024 queries per core
NQT = NQC // 128  # 8 query tiles per core
NKT = NK // 128  # 16 key tiles
RMS_EPS = 1.1920929e-07

_CACHE = {}


def _build_nc():
    import concourse.bacc as bacc
    import concourse.mybir as mybir
    import concourse.tile as tile

    fp32 = mybir.dt.float32
    f16 = mybir.dt.float16

    nc = bacc.Bacc("TRN2", target_bir_lowering=False, debug=False)

    q_d = nc.dram_tensor("q", [NQC, D], fp32, kind="ExternalInput").ap()
    k_d = nc.dram_tensor("k", [NK, D], fp32, kind="ExternalInput").ap()
    v_d = nc.dram_tensor("v", [NK, DV], f16, kind="ExternalInput").ap()
    wq_d = nc.dram_tensor("wqt", [D, D], f16, kind="ExternalInput").ap()
    wk_d = nc.dram_tensor("wkt", [D, D], f16, kind="ExternalInput").ap()
    bq_d = nc.dram_tensor("bqe", [D], fp32, kind="ExternalInput").ap()
    bk_d = nc.dram_tensor("bke", [D], fp32, kind="ExternalInput").ap()
    o_d = nc.dram_tensor("o", [NQC, DV], fp32, kind="ExternalOutput").ap()

    with tile.TileContext(nc) as tc:
        _tile_kernel(tc, o_d, q_d, k_d, v_d, wq_d, wk_d, bq_d, bk_d)
    nc.compile()
    return nc


def _tile_kernel(tc, o_d, q_d, k_d, v_d, wq_d, wk_d, bq_d, bk_d):
    from contextlib import ExitStack

    import concourse.mybir as mybir

    nc = tc.nc
    fp32 = mybir.dt.float32
    f16 = mybir.dt.float16
    AF = mybir.ActivationFunctionType
    OP = mybir.AluOpType
    AX = mybir.AxisListType

    ctx = ExitStack()
    with ctx:
        singles = ctx.enter_context(tc.tile_pool(name="singles", bufs=1))

        # resident tensors
        v_sb = singles.tile([128, NKT, DV], f16)
        nc.sync.dma_start(out=v_sb, in_=v_d.rearrange("(c p) d -> p c d", p=128))
        wq_sb = singles.tile([128, D], f16)
        nc.sync.dma_start(out=wq_sb, in_=wq_d)
        wk_sb = singles.tile([128, D], f16)
        nc.sync.dma_start(out=wk_sb, in_=wk_d)
        bq_sb = singles.tile([128, 1], fp32)
        nc.sync.dma_start(out=bq_sb, in_=bq_d[:, None])
        bk_sb = singles.tile([128, 1], fp32)
        nc.sync.dma_start(out=bk_sb, in_=bk_d[:, None])

        qx_sb = singles.tile([128, NQT, D], fp32)
        nc.sync.dma_start(out=qx_sb, in_=q_d.rearrange("(c p) d -> p c d", p=128))
        kx_sb = singles.tile([128, NKT, D], fp32)
        nc.sync.dma_start(out=kx_sb, in_=k_d.rearrange("(c p) d -> p c d", p=128))

        eps_sb = singles.tile([128, 1], fp32)
        nc.vector.memset(eps_sb, RMS_EPS)

        qxT = singles.tile([128, NQC], f16)  # normalized, transposed [d, tok]
        kxT = singles.tile([128, NK], f16)
        qT = singles.tile([128, NQC], f16)  # projected (per-head rows 32h..32h+31)
        kT = singles.tile([128, NK], f16)

        # ---- preamble: RMSNorm + transpose + projections ----
        with (
            tc.tile_pool(name="pre", bufs=2) as pre,
            tc.tile_pool(name="prepsum", bufs=2, space="PSUM") as prepsum,
        ):
            for x_sb, nt, xT in ((qx_sb, NQT, qxT), (kx_sb, NKT, kxT)):
                n = nt * D
                sq = pre.tile([128, nt, D], fp32, tag="sq", bufs=1)
                nc.vector.tensor_mul(sq, x_sb, x_sb)
                ssq = pre.tile([128, nt], fp32, tag="ssq", bufs=1)
                nc.vector.tensor_reduce(ssq[:, :, None], sq, AX.X, OP.add)
                sd = pre.tile([128, nt], fp32, tag="sd", bufs=1)
                # sd = sqrt(mean(x^2) + eps)  (bias AP is per-partition)
                nc.scalar.activation(sd, ssq, AF.Sqrt, bias=eps_sb, scale=1.0 / D)
                rstd = pre.tile([128, nt], fp32, tag="rstd", bufs=1)
                nc.vector.reciprocal(rstd, sd)
                xn = pre.tile([128, nt, D], f16, tag="xn", bufs=1)
                for t in range(nt):
                    nc.vector.tensor_scalar_mul(
                        xn[:, t, :], x_sb[:, t, :], rstd[:, t : t + 1]
                    )
                # one xbar transpose: out[p, c, j] = xn_flat[j, c*128+p]
                # -> xT block c holds (xn tile c)^T, i.e. xT[d, tok] overall.
                nc.sync.dma_start_transpose(
                    out=xT.rearrange("p (c j) -> p c j", j=128),
                    in_=xn.rearrange("p c j -> p (c j)"),
                )
            for xT, n512, w_sb, b_sb, dst in (
                (qxT, NQC // 512, wq_sb, bq_sb, qT),
                (kxT, NK // 512, wk_sb, bk_sb, kT),
            ):
                for j in range(n512):
                    pp = prepsum.tile([128, 512], fp32, tag="proj")
                    nc.tensor.matmul(
                        pp,
                        lhsT=w_sb,
                        rhs=xT[:, j * 512 : (j + 1) * 512],
                        start=True,
                        stop=True,
                    )
                    nc.scalar.activation(
                        dst[:, j * 512 : (j + 1) * 512], pp, AF.Identity, bias=b_sb
                    )

        # ---- main loop over query tiles ----
        with (
            tc.tile_pool(name="spsum", bufs=2, space="PSUM") as spool,
            tc.tile_pool(name="opsum", bufs=2, space="PSUM") as opool,
            tc.tile_pool(name="pwork", bufs=2) as pwork,
            tc.tile_pool(name="awork", bufs=2) as awork,
            tc.tile_pool(name="owork", bufs=2) as owork,
            tc.tile_pool(name="small", bufs=2) as small,
        ):
            for qc in range(NQT):
                qsl = slice(qc * 128, (qc + 1) * 128)
                P = pwork.tile([128, H, NK], f16, tag="P")
                racc = small.tile([128, H, 2], fp32, tag="racc")
                for h in range(H):
                    for half in range(2):
                        S = spool.tile([128, 1024], fp32, tag="S")  # 2 banks
                        for kc in range(2):
                            ko = half * 1024 + kc * 512
                            nc.tensor.matmul(
                                S[:, kc * 512 : (kc + 1) * 512],
                                lhsT=qT[32 * h : 32 * (h + 1), qsl],
                                rhs=kT[32 * h : 32 * (h + 1), ko : ko + 512],
                                start=True,
                                stop=True,
                                tile_position=(32 * h, 0),
                            )
                        nc.scalar.activation(
                            P[:, h, half * 1024 : (half + 1) * 1024],
                            S,
                            AF.Exp,
                            accum_out=racc[:, h, half : half + 1],
                        )
                # 1/R per head
                rsum = small.tile([128, H], fp32, tag="rsum")
                nc.vector.tensor_add(rsum, racc[:, :, 0], racc[:, :, 1])
                crec = small.tile([128, H], fp32, tag="crec")
                nc.vector.reciprocal(crec, rsum)
                # A = sum_h P_h / R_h   (TS at 4x + TT tree-adds at 2x)
                t1 = awork.tile([128, NK], f16, tag="t1")
                t2 = awork.tile([128, NK], f16, tag="t2")
                t3 = awork.tile([128, NK], f16, tag="t3")
                A = awork.tile([128, NK], f16, tag="A")
                nc.vector.tensor_scalar_mul(A, P[:, 0, :], crec[:, 0:1])
                nc.vector.tensor_scalar_mul(t1, P[:, 1, :], crec[:, 1:2])
                nc.vector.tensor_scalar_mul(t2, P[:, 2, :], crec[:, 2:3])
                nc.vector.tensor_scalar_mul(t3, P[:, 3, :], crec[:, 3:4])
                nc.vector.tensor_add(t2, t2, t3)
                nc.vector.tensor_add(A, A, t1)
                nc.vector.tensor_add(A, A, t2)
                # A^T in one xbar transpose: AT block kc = (A cols kc*128..)^T
                AT = awork.tile([128, NK], f16, tag="AT")
                nc.sync.dma_start_transpose(
                    out=AT.rearrange("p (c j) -> p c j", j=128), in_=A
                )
                # O = A @ V  (V pre-scaled by 0.25)
                O = opool.tile([128, DV], fp32, tag="O")
                for kc in range(NKT):
                    st, sp = kc == 0, kc == NKT - 1
                    lhsT = AT[:, kc * 128 : (kc + 1) * 128]
                    nc.tensor.matmul(
                        O[:, 0:512], lhsT=lhsT, rhs=v_sb[:, kc, 0:512],
                        start=st, stop=sp,
                    )
                    nc.tensor.matmul(
                        O[:, 512:DV], lhsT=lhsT, rhs=v_sb[:, kc, 512:DV],
                        start=st, stop=sp,
                    )
                osb = owork.tile([128, DV], fp32, tag="osb")
                nc.vector.tensor_copy(osb, O)
                nc.sync.dma_start(out=o_d[qsl, :], in_=osb)


def _get_nc():
    if "nc" not in _CACHE:
        _CACHE["nc"] = _build_nc()
    return _CACHE["nc"]


def _host_prep(query, key, value, rms_q_w, rms_k_w, Wq, Wk, bq, bk):
    s = np.sqrt(float(HD))
    wqt = (rms_q_w[:, None] * Wq.T / s).astype(np.float16)
    wkt = (rms_k_w[:, None] * Wk.T).astype(np.float16)
    bqe = (bq / s).astype(np.float32)
    bke = bk.astype(np.float32)
    vq = (0.25 * value).astype(np.float16)  # [B, NK, DV]
    in_maps = []
    nq_blk = NQ // (N_CORES // B)  # 1024
    for c in range(N_CORES):
        b, qi = divmod(c, N_CORES // B)
        in_maps.append(
            {
                "q": np.ascontiguousarray(
                    query[b, qi * nq_blk : (qi + 1) * nq_blk]
                ).astype(np.float32),
                "k": np.ascontiguousarray(key[b]).astype(np.float32),
                "v": np.ascontiguousarray(vq[b]),
                "wqt": wqt,
                "wkt": wkt,
                "bqe": bqe,
                "bke": bke,
            }
        )
    return in_maps


def kernel(query, key, value, rms_q_w, rms_k_w, Wq, Wk, bq, bk, _trace=False):
    from concourse import bass_utils

    in_maps = _host_prep(
        np.asarray(query), np.asarray(key), np.asarray(value),
        np.asarray(rms_q_w), np.asarray(rms_k_w),
        np.asarray(Wq), np.asarray(Wk), np.asarray(bq), np.asarray(bk),
    )
    nc = _get_nc()
    res = bass_utils.run_bass_kernel_spmd(
        nc, in_maps, core_ids=list(range(N_CORES)), trace=_trace
    )
    _CACHE["last_results"] = res
    outs = [np.asarray(r["o"], dtype=np.float32) for r in res.results]
    nq_blk = NQ // (N_CORES // B)
    out = np.empty((B, NQ, DV), dtype=np.float32)
    for c in range(N_CORES):
        b, qi = divmod(c, N_CORES // B)
        out[b, qi * nq_blk : (qi + 1) * nq_blk] = outs[c]
    return out
